# revision 12
# baseline (speedup 1.0000x reference)
# Laplacian normalization kernel for Trainium2 (8 NeuronCores, SPMD).
#
# out = d^-1/2[:, None] * A * d^-1/2[None, :],  d_i = sum_j A[i, j],  A: [8192, 8192] f32
#
# Data path in bf16 (gate is rel_err < 2e-2; measured ~1.3%): host casts A,
# device reads/writes bf16, host upcasts. 16MB in + 16MB out per core at the
# ~355 GB/s per-core DMA cap -> ~95us of unavoidable DMA time. The baseline
# (load all -> one AllGather -> store all) exposed the whole collective
# window (~33us of DMA-idle) plus a slow store ramp; this version pipelines
# stores against loads and the collectives.
#
# Block-interleaved row sharding: core c owns rows
#   { b*2048 + c*256 + r : r < 256, b < 4 }   (4 blocks = local tiles 2b, 2b+1)
# so the stage-b AllGather of the 8 cores' block-b d^-1/2 (512B in, 4KB out)
# yields the CONTIGUOUS global slice d^-1/2[b*2048:(b+1)*2048] == the cvec
# columns for stage b, full-width stores for EVERY loaded tile. Stores start
# right after AG0 lands and overlap the remaining loads + later AGs; the
# only AG-exposed tail is stage 3.
#
# The Tile scheduler is a sim-driven list scheduler: DMA dispatches are
# always-ready and get hoisted above not-yet-ready compute on the same
# engine, where ring-depth (4) blocking then stalls that engine (measured
# +14us on ACT's first reduce in a previous revision). Only SP, ACT and
# gpsimd can start DMAs here, so:
#   sync ring : loads t0/t2/t4 (2MB ops), then the 4 cvec broadcast fetches
#         (dram -> [128,2048] SBUF, stage-gated in stage order), then ALL
#         DVE-produced stores (SP has no compute to stall).
#   SWDGE ring: loads t1/t3/t5/t6/t7, then gpsimd's 8 stores.
#   scalar ring: exactly 4 dloc AllGather-input writes (= ring depth, so
#         the hoisted dispatches can never block ACT's reduces).
#   ACT : row-sum reduces only - full tile for evens 0/2/4, second half for
#         odds and for both stage-3 tiles. Copy table only, never switched.
#   DVE : first-half reduces + combines; d^-1/2 per stage via the
#         0x5f3759df bit-trick + 2 Newton steps (~5e-6 rel err; keeps ACT's
#         Sqrt table out); PSUM->SBUF transpose downcast; row scales
#         (tensor_scalar 4x); col-scale chunks (tensor_tensor 2x), with
#         stage pairs merged into single [128,4096] ops / 1MB stores where
#         both stages' cvec halves are ready by then.
#   PE  : transpose [128,2] -> [2,128] per stage.
#   gpsimd: engine col-scales tiles 0/1 (one chunk per stage) to prime the
#         store pipe while DVE is still on the reduce chain.
#   Collective triggers are queue-side (measured: they do not block the
#   gpsimd engine), and no warmup AllGather: AG0's input (~23us) is ready
#   before the cc-stream init + barrier (~32us) completes anyway, so a
#   warmup could only delay AG0.

import numpy as np

N = 8192
NCORES = 8
R = N // NCORES   # 1024 rows per core
P = 128           # SBUF partitions
T = R // P        # 8 row-tiles of [128, 8192] per core
NST = 4           # AllGather stages (2 tiles each)
SW = N // NST     # stage column width (2048)
BR = 2 * P        # rows per core per block (256)
W = N // 2        # half-tile width (4096 cols) for the split reduces

RSQRT_MAGIC = 0x5F3759E0  # 0x5f3759df + 1 (K - x == ~x + (K + 1))

_cache = {}


def _build():
    import concourse.bacc as bacc
    import concourse.mybir as mybir
    import concourse.tile as tile
    from concourse import masks

    f32 = mybir.dt.float32
    i32 = mybir.dt.int32
    bf16 = mybir.dt.bfloat16
    X = mybir.AxisListType.X
    mult = mybir.AluOpType.mult
    add = mybir.AluOpType.add
    lsr = mybir.AluOpType.logical_shift_right
    xor = mybir.AluOpType.bitwise_xor
    Copy = mybir.ActivationFunctionType.Copy

    nc = bacc.Bacc(
        "TRN2", target_bir_lowering=False, debug=False, num_devices=NCORES
    )
    a = nc.dram_tensor("a_shard", [R, N], bf16, kind="ExternalInput").ap()
    out = nc.dram_tensor("out_shard", [R, N], bf16, kind="ExternalOutput").ap()

    a_t = a.rearrange("(t p) n -> t p n", p=P)
    o_t = out.rearrange("(t p) n -> t p n", p=P)

    with tile.TileContext(nc) as tc:
        with (
            tc.tile_pool(name="cpool", bufs=1) as cpool,
            tc.tile_pool(name="vpool", bufs=1) as vpool,
            tc.tile_pool(name="psum", bufs=2, space="PSUM") as psum,
            tc.tile_pool(name="dram", bufs=1, space="DRAM") as dram,
        ):
            tiles = []
            for t in range(T):
                big = cpool.tile([P, N], bf16, tag=f"c{t}")
                tiles.append(big)
            dsum = vpool.tile([P, T], f32, tag="dsum")
            dinv = vpool.tile([P, T], f32, tag="dinv")
            hp = vpool.tile([P, T], f32, tag="hp")
            nsc = vpool.tile([P, T], f32, tag="nsc")
            cvec = vpool.tile([P, N], bf16, tag="cvec")
            dinv_tps = []
            for b in range(NST):
                dtp = vpool.tile([2, P], bf16, tag=f"dtp{b}")
                dinv_tps.append(dtp)
            ident = vpool.tile([P, P], f32, tag="ident")
            dloc = dram.tile([NST, BR], bf16, tag="dloc")
            dfull = dram.tile([NST, SW], bf16, tag="dfull")

            groups = [list(range(NCORES))]

            # --- t=0: loads. sync ring gets t0/t2/t4 (it later carries the
            # stage-gated cvec fetches + stores); SWDGE the other five.
            for t in (0, 2, 4):
                nc.sync.dma_start(out=tiles[t][:, :], in_=a_t[t][:, :])
            for t in (1, 3, 5, 6, 7):
                nc.gpsimd.dma_start(out=tiles[t][:, :], in_=a_t[t][:, :])
            masks.make_identity(nc, ident[:, :])

            # --- ACT: pure reduce chain (no dispatches that could block).
            for t in range(T):
                if t in (0, 2, 4):
                    nc.scalar.activation(
                        out=tiles[t][:, :], in_=tiles[t][:, :], func=Copy,
                        accum_out=dsum[:, t : t + 1],
                    )
                else:
                    nc.scalar.activation(
                        out=tiles[t][:, W:N], in_=tiles[t][:, W:N], func=Copy,
                        accum_out=hp[:, t : t + 1],
                    )

            # --- DVE helpers ---
            def rsqrt_stage(b):
                """dinv[:, 2b:2b+2] = dsum[:, 2b:2b+2] ** -0.5 on DVE."""
                d = dsum[:, 2 * b : 2 * b + 2]
                y = dinv[:, 2 * b : 2 * b + 2]
                tt = nsc[:, 2 * b : 2 * b + 2]
                nc.vector.tensor_scalar(
                    out=y.bitcast(i32), in0=d.bitcast(i32),
                    scalar1=1, scalar2=-1, op0=lsr, op1=xor,
                )
                nc.vector.tensor_scalar(
                    out=y.bitcast(i32), in0=y.bitcast(i32),
                    scalar1=RSQRT_MAGIC, scalar2=None, op0=add,
                )
                for _ in range(2):
                    nc.vector.tensor_tensor(out=tt, in0=y, in1=y, op=mult)
                    nc.vector.tensor_tensor(out=tt, in0=tt, in1=d, op=mult)
                    nc.vector.tensor_scalar(
                        out=tt, in0=tt, scalar1=-0.5, scalar2=1.5,
                        op0=mult, op1=add,
                    )
                    nc.vector.tensor_tensor(out=y, in0=y, in1=tt, op=mult)

            def dve_half_red(t):
                nc.vector.reduce_sum(
                    out=nsc[:, t : t + 1], in_=tiles[t][:, 0:W], axis=X
                )

            def dve_comb(t):
                nc.vector.tensor_tensor(
                    out=dsum[:, t : t + 1], in0=nsc[:, t : t + 1],
                    in1=hp[:, t : t + 1], op=add,
                )

            def rowscale(t):
                nc.vector.tensor_scalar(
                    out=tiles[t][:, :], in0=tiles[t][:, :],
                    scalar1=dinv[:, t : t + 1], scalar2=None, op0=mult,
                )

            def dve_col(c0, c1, t):
                """col-scale tiles[t][:, c0:c1] on DVE + store on sync ring."""
                cols = slice(c0, c1)
                nc.vector.tensor_tensor(
                    out=tiles[t][:, cols], in0=tiles[t][:, cols],
                    in1=cvec[:, cols], op=mult,
                )
                nc.sync.dma_start(out=o_t[t][:, cols], in_=tiles[t][:, cols])

            def stage_pack(b):
                """rsqrt, transpose, dloc write (scalar ring), AllGather
                trigger, cvec broadcast fetch (sync ring, stage-gated)."""
                rsqrt_stage(b)
                tp = psum.tile([2, P], f32, tag=f"tp{b % 2}")
                nc.tensor.transpose(
                    tp[:, :], dinv[:, 2 * b : 2 * b + 2], ident[:, :]
                )
                nc.vector.tensor_copy(out=dinv_tps[b][:, :], in_=tp[:, :])
                nc.scalar.dma_start(out=dloc[b, :], in_=dinv_tps[b][:, :])
                nc.gpsimd.collective_compute(
                    "AllGather",
                    mybir.AluOpType.bypass,
                    replica_groups=groups,
                    ins=[dloc[b : b + 1, :].opt()],
                    outs=[dfull[b : b + 1, :].opt()],
                )
                cols = slice(b * SW, (b + 1) * SW)
                nc.sync.dma_start(
                    out=cvec[:, cols],
                    in_=dfull[b : b + 1, :].to_broadcast((P, SW)),
                )

            # --- DVE program ---
            dve_half_red(1)
            dve_comb(1)
            stage_pack(0)
            rowscale(0)
            rowscale(1)
            dve_half_red(3)
            dve_comb(3)
            stage_pack(1)
            rowscale(2)
            rowscale(3)
            dve_half_red(5)
            dve_comb(5)
            stage_pack(2)
            dve_half_red(6)
            dve_col(0, SW, 2)
            dve_half_red(7)
            dve_comb(6)
            dve_comb(7)
            stage_pack(3)
            dve_col(0, SW, 3)
            rowscale(4)
            rowscale(5)
            dve_col(0, 2 * SW, 4)       # tiles 4-7: stages 0+1 as one 1MB op
            dve_col(0, 2 * SW, 5)
            rowscale(6)
            rowscale(7)
            dve_col(0, 2 * SW, 6)
            dve_col(0, 2 * SW, 7)
            dve_col(SW, 2 * SW, 2)
            dve_col(SW, 2 * SW, 3)
            for t in range(2, T):       # stages 2+3 as one 1MB op
                dve_col(2 * SW, N, t)

            # --- gpsimd: col-scale tiles 0/1 per stage (primes the store
            # pipe while DVE is on the chain); stores on its SWDGE ring.
            for s in range(NST):
                for t in (0, 1):
                    cols = slice(s * SW, (s + 1) * SW)
                    nc.gpsimd.tensor_tensor(
                        out=tiles[t][:, cols], in0=tiles[t][:, cols],
                        in1=cvec[:, cols], op=mult,
                    )
                    nc.gpsimd.dma_start(
                        out=o_t[t][:, cols], in_=tiles[t][:, cols]
                    )

    nc.compile()
    return nc


def _core_rows(c):
    return np.concatenate(
        [np.arange(b * 2048 + c * BR, b * 2048 + (c + 1) * BR) for b in range(NST)]
    )


def kernel(adjacency_matrix, _trace=False):
    import ml_dtypes
    from concourse.bass_utils import run_bass_kernel_spmd

    A = np.asarray(adjacency_matrix)
    assert A.shape == (N, N), A.shape
    A_bf = A.astype(ml_dtypes.bfloat16)

    if "nc" not in _cache:
        _cache["nc"] = _build()
    nc = _cache["nc"]

    in_maps = [
        {"a_shard": np.ascontiguousarray(A_bf[_core_rows(c)])} for c in range(NCORES)
    ]
    res = run_bass_kernel_spmd(
        nc, in_maps, core_ids=list(range(NCORES)), trace=_trace
    )
    _cache["last"] = res
    out = np.empty((N, N), dtype=np.float32)
    for c in range(NCORES):
        shard = res.results[c]["out_shard"]
        for b in range(NST):
            out[b * 2048 + c * BR : b * 2048 + (c + 1) * BR] = shard[
                b * BR : (b + 1) * BR
            ]
    return out


# revision 15
# speedup vs baseline: 1.0426x; 1.0426x over previous
# Laplacian normalization kernel for Trainium2 (8 NeuronCores, SPMD).
#
# out = d^-1/2[:, None] * A * d^-1/2[None, :],  d_i = sum_j A[i, j],  A: [8192, 8192] f32
#
# Data path in bf16 (gate is rel_err < 2e-2; measured ~1.3%): host casts A,
# device reads/writes bf16, host upcasts. 16MB in + 16MB out per core at the
# ~355 GB/s per-core DMA cap -> ~95us of unavoidable DMA time. The previous
# version (load all -> one AllGather -> store all) exposed the whole
# collective window (~33us of DMA-idle) plus a slow store ramp; this version
# pipelines stores against loads and the collectives.
#
# Block-interleaved row sharding: core c owns rows
#   { b*2048 + c*256 + r : r < 256, b < 4 }   (4 blocks = local tiles 2b, 2b+1)
# so the stage-b AllGather of the 8 cores' block-b d^-1/2 (512B in, 4KB out)
# yields the CONTIGUOUS global slice d^-1/2[b*2048:(b+1)*2048] == the cvec
# columns for stage b, full-width stores for EVERY loaded tile. Stores start
# right after AG0 lands (~46us, while tiles 6/7 still load) and the only
# AG-exposed tail is stage 3 (~4MB after ~66us).
#
# Engine/queue layout (the Tile scheduler hoists always-ready DMA dispatches
# above not-yet-ready compute on the same engine, and ring depth is 4, so
# placement is chosen to keep deadline engines from ever blocking):
#   loads: 1MB half-tile chunks alternating the sync/scalar HWDGE rings,
#     dispatched with 2 tiles of lookahead woven between ACT reduces
#     (dispatching everything up front blocks ACT on ring space); tiles
#     0/1/7 go in 2048-col quarters (faster ramp / shorter reduce tail).
#     t3h0+t5h0 ride the SWDGE ring so gpsimd can reduce them early.
#   ACT: full-tile reduces (Copy+accum, in place) for tiles 0/2/4/6 and
#     t7's last quarter. Copy table only, never switched.
#   gpsimd engine: half-reduces of t3h0/t5h0 (~5.7us each, in its idle
#     window) then col-scales tiles 0/1 per stage (primes the store pipe
#     while DVE is on its chain).
#   DVE: t1 full reduce (halves+combine), h1 halves of t3/t5, t7 quarters,
#     combines; per-stage d^-1/2 via the 0x5f3759df bit-trick + 2 Newton
#     steps (~5e-6 rel err; keeps ACT's Sqrt table out); PSUM->SBUF
#     transpose downcast; row scales (4x); the remaining col-scale chunks.
#   PE: transpose [128,2] -> [2,128] per stage.
#   SWDGE ring also carries dloc/cvec-broadcast control DMAs interleaved in
#     readiness order with gpsimd's stores (a bulk HWDGE ring would head-of
#     -line block them behind queued loads/stores).
#   DVE-produced stores alternate the sync/scalar rings (positioned after
#     all load chunks, matching both ring drain and DVE production order).
#   A warmup AllGather at t=0 absorbs the cc-stream init + barrier +
#     first-op penalty (~30us, measured), so the 4 stage AllGathers run
#     back-to-back at ~5-6us each as their inputs land.

import numpy as np

N = 8192
NCORES = 8
R = N // NCORES   # 1024 rows per core
P = 128           # SBUF partitions
T = R // P        # 8 row-tiles of [128, 8192] per core
NST = 4           # AllGather stages (2 tiles each)
SW = N // NST     # stage column width (2048)
BR = 2 * P        # rows per core per block (256)
W = N // 2        # half-tile width (4096 cols)
Q = N // 4        # quarter width (2048 cols)

RSQRT_MAGIC = 0x5F3759E0  # 0x5f3759df + 1 (K - x == ~x + (K + 1))

GP_LOADS = ((3, 0), (5, 0))  # (tile, half) pairs loaded via SWDGE

_cache = {}


def _build():
    import concourse.bacc as bacc
    import concourse.mybir as mybir
    import concourse.tile as tile
    from concourse import masks

    f32 = mybir.dt.float32
    i32 = mybir.dt.int32
    bf16 = mybir.dt.bfloat16
    X = mybir.AxisListType.X
    mult = mybir.AluOpType.mult
    add = mybir.AluOpType.add
    lsr = mybir.AluOpType.logical_shift_right
    xor = mybir.AluOpType.bitwise_xor
    Copy = mybir.ActivationFunctionType.Copy

    nc = bacc.Bacc(
        "TRN2", target_bir_lowering=False, debug=False, num_devices=NCORES
    )
    a = nc.dram_tensor("a_shard", [R, N], bf16, kind="ExternalInput").ap()
    out = nc.dram_tensor("out_shard", [R, N], bf16, kind="ExternalOutput").ap()

    a_t = a.rearrange("(t p) n -> t p n", p=P)
    o_t = out.rearrange("(t p) n -> t p n", p=P)

    with tile.TileContext(nc) as tc:
        with (
            tc.tile_pool(name="cpool", bufs=1) as cpool,
            tc.tile_pool(name="vpool", bufs=1) as vpool,
            tc.tile_pool(name="psum", bufs=2, space="PSUM") as psum,
            tc.tile_pool(name="dram", bufs=1, space="DRAM") as dram,
        ):
            tiles = []
            for t in range(T):
                big = cpool.tile([P, N], bf16, tag=f"c{t}")
                tiles.append(big)
            dsum = vpool.tile([P, T], f32, tag="dsum")
            dinv = vpool.tile([P, T], f32, tag="dinv")
            hp = vpool.tile([P, 16], f32, tag="hp")
            nsc = vpool.tile([P, T], f32, tag="nsc")
            cvec = vpool.tile([P, N], bf16, tag="cvec")
            dinv_tps = []
            for b in range(NST):
                dtp = vpool.tile([2, P], bf16, tag=f"dtp{b}")
                dinv_tps.append(dtp)
            ident = vpool.tile([P, P], f32, tag="ident")
            wsrc = vpool.tile([1, 8], bf16, tag="wsrc")
            dloc = dram.tile([NST, BR], bf16, tag="dloc")
            dfull = dram.tile([NST, SW], bf16, tag="dfull")
            warm = dram.tile([1, 8], bf16, tag="warm")
            warm_o = dram.tile([1, 8 * NCORES], bf16, tag="warm_o")

            groups = [list(range(NCORES))]

            def allgather(i, o):
                nc.gpsimd.collective_compute(
                    "AllGather",
                    mybir.AluOpType.bypass,
                    replica_groups=groups,
                    ins=[i.opt()],
                    outs=[o.opt()],
                )

            # --- t=0: SWDGE early loads, then warmup collective, identity.
            for t, h in GP_LOADS:
                cols = slice(h * W, (h + 1) * W)
                nc.gpsimd.dma_start(out=tiles[t][:, cols], in_=a_t[t][:, cols])
            nc.vector.memset(wsrc[:, :], 0.0)
            nc.gpsimd.dma_start(out=warm[0, :], in_=wsrc[0, :])
            allgather(warm[0, :], warm_o[0, :])
            masks.make_identity(nc, ident[:, :])



            # --- HWDGE load weave (baseline-proven): 2 tiles of lookahead,
            # chunks alternate the sync/scalar rings; tiles 0/1/7 load in
            # quarters.
            ld = [nc.sync, nc.scalar]
            nld = [0]

            def hw_loads(t):
                nch = 4 if t in (0, 1, T - 1) else 2
                w = N // nch
                for h in range(nch):
                    if (t, h * w // W) in GP_LOADS and w == W:
                        continue
                    cols = slice(h * w, (h + 1) * w)
                    ld[nld[0] % 2].dma_start(
                        out=tiles[t][:, cols], in_=a_t[t][:, cols]
                    )
                    nld[0] += 1

            def act_red_full(t):
                nc.scalar.activation(
                    out=tiles[t][:, :], in_=tiles[t][:, :], func=Copy,
                    accum_out=dsum[:, t : t + 1],
                )

            # --- DVE helpers ---
            def rsqrt_stage(b):
                d = dsum[:, 2 * b : 2 * b + 2]
                y = dinv[:, 2 * b : 2 * b + 2]
                tt = nsc[:, 2 * b : 2 * b + 2]
                nc.vector.tensor_scalar(
                    out=y.bitcast(i32), in0=d.bitcast(i32),
                    scalar1=1, scalar2=-1, op0=lsr, op1=xor,
                )
                nc.vector.tensor_scalar(
                    out=y.bitcast(i32), in0=y.bitcast(i32),
                    scalar1=RSQRT_MAGIC, scalar2=None, op0=add,
                )
                for _ in range(2):
                    nc.vector.tensor_tensor(out=tt, in0=y, in1=y, op=mult)
                    nc.vector.tensor_tensor(out=tt, in0=tt, in1=d, op=mult)
                    nc.vector.tensor_scalar(
                        out=tt, in0=tt, scalar1=-0.5, scalar2=1.5,
                        op0=mult, op1=add,
                    )
                    nc.vector.tensor_tensor(out=y, in0=y, in1=tt, op=mult)

            def stage_pack(b):
                """rsqrt -> PE transpose -> bf16 downcast -> dloc (SWDGE) ->
                AllGather -> cvec broadcast fetch (SWDGE)."""
                rsqrt_stage(b)
                tp = psum.tile([2, P], f32, tag=f"tp{b % 2}")
                nc.tensor.transpose(
                    tp[:, :], dinv[:, 2 * b : 2 * b + 2], ident[:, :]
                )
                nc.vector.tensor_copy(out=dinv_tps[b][:, :], in_=tp[:, :])
                nc.gpsimd.dma_start(out=dloc[b, :], in_=dinv_tps[b][:, :])
                allgather(dloc[b : b + 1, :], dfull[b : b + 1, :])

            def cvb(b):
                cols = slice(b * SW, (b + 1) * SW)
                nc.gpsimd.dma_start(
                    out=cvec[:, cols],
                    in_=dfull[b : b + 1, :].to_broadcast((P, SW)),
                )

            def rowscale(t):
                nc.vector.tensor_scalar(
                    out=tiles[t][:, :], in0=tiles[t][:, :],
                    scalar1=dinv[:, t : t + 1], scalar2=None, op0=mult,
                )

            st_rings = [nc.sync, nc.scalar]
            st_n = [0]

            def dve_col(c0, c1, t):
                cols = slice(c0, c1)
                nc.vector.tensor_tensor(
                    out=tiles[t][:, cols], in0=tiles[t][:, cols],
                    in1=cvec[:, cols], op=mult,
                )
                st_rings[st_n[0] % 2].dma_start(
                    out=o_t[t][:, cols], in_=tiles[t][:, cols]
                )
                st_n[0] += 1

            # --- phase A: loads woven with reduces, stage packs as soon as
            # each stage's two row-sums complete.
            hw_loads(0)
            hw_loads(1)
            # tile 0 (ACT full) + tile 1 (DVE halves)
            hw_loads(2)
            act_red_full(0)
            nc.vector.reduce_sum(out=hp[:, 1:2], in_=tiles[1][:, 0:W], axis=X)
            nc.vector.reduce_sum(out=hp[:, 2:3], in_=tiles[1][:, W:N], axis=X)
            nc.vector.tensor_tensor(
                out=dsum[:, 1:2], in0=hp[:, 1:2], in1=hp[:, 2:3], op=add
            )
            stage_pack(0)
            cvb(0)
            rowscale(0)
            rowscale(1)
            # tile 2 (ACT) + tile 3 (DVE h0 [SWDGE, lands ~15us] + ACT h1)
            hw_loads(3)
            act_red_full(2)
            nc.vector.reduce_sum(out=nsc[:, 3:4], in_=tiles[3][:, 0:W], axis=X)
            nc.scalar.activation(
                out=tiles[3][:, W:N], in_=tiles[3][:, W:N], func=Copy,
                accum_out=hp[:, 3:4],
            )
            nc.vector.tensor_tensor(
                out=dsum[:, 3:4], in0=nsc[:, 3:4], in1=hp[:, 3:4], op=add
            )
            stage_pack(1)
            cvb(1)
            rowscale(2)
            rowscale(3)
            # tile 4 (ACT) + tile 5 (DVE h0 [SWDGE] + ACT h1)
            hw_loads(4)
            act_red_full(4)
            hw_loads(5)
            nc.vector.reduce_sum(out=nsc[:, 5:6], in_=tiles[5][:, 0:W], axis=X)
            nc.scalar.activation(
                out=tiles[5][:, W:N], in_=tiles[5][:, W:N], func=Copy,
                accum_out=hp[:, 5:6],
            )
            nc.vector.tensor_tensor(
                out=dsum[:, 5:6], in0=nsc[:, 5:6], in1=hp[:, 5:6], op=add
            )
            stage_pack(2)
            cvb(2)
            # tile 6 (ACT) + tile 7 (DVE 3 quarters + ACT last quarter)
            hw_loads(6)
            act_red_full(6)
            hw_loads(7)
            for qi in range(3):
                nc.vector.reduce_sum(
                    out=hp[:, 9 + qi : 10 + qi],
                    in_=tiles[7][:, qi * Q : (qi + 1) * Q],
                    axis=X,
                )
            nc.scalar.activation(
                out=tiles[7][:, 3 * Q : N], in_=tiles[7][:, 3 * Q : N],
                func=Copy, accum_out=hp[:, 12:13],
            )
            rowscale(4)
            nc.vector.reduce_sum(out=dsum[:, 7:8], in_=hp[:, 9:13], axis=X)
            stage_pack(3)
            cvb(3)

            # --- DVE tail: col chunks; stage 0 chunks woven first, tiles
            # 4-7 merge stages 0+1 into one [128,4096] op / 1MB store.
            dve_col(0, SW, 2)
            rowscale(5)
            dve_col(0, SW, 3)
            rowscale(6)
            rowscale(7)
            dve_col(0, 2 * SW, 4)
            dve_col(0, 2 * SW, 5)
            dve_col(0, 2 * SW, 6)
            dve_col(0, 2 * SW, 7)
            dve_col(SW, 2 * SW, 2)
            dve_col(SW, 2 * SW, 3)
            for t in range(2, T):
                dve_col(2 * SW, 3 * SW, t)
            for t in range(2, T):
                dve_col(3 * SW, N, t)

            # --- gpsimd: col-scale tiles 0/1 per stage; stores on SWDGE.
            for s in range(NST):
                for t in (0, 1):
                    cols = slice(s * SW, (s + 1) * SW)
                    nc.gpsimd.tensor_tensor(
                        out=tiles[t][:, cols], in0=tiles[t][:, cols],
                        in1=cvec[:, cols], op=mult,
                    )
                    nc.gpsimd.dma_start(
                        out=o_t[t][:, cols], in_=tiles[t][:, cols]
                    )

    nc.compile()
    return nc


def _core_rows(c):
    return np.concatenate(
        [np.arange(b * 2048 + c * BR, b * 2048 + (c + 1) * BR) for b in range(NST)]
    )


def kernel(adjacency_matrix, _trace=False):
    import ml_dtypes
    from concourse.bass_utils import run_bass_kernel_spmd

    A = np.asarray(adjacency_matrix)
    assert A.shape == (N, N), A.shape
    A_bf = A.astype(ml_dtypes.bfloat16)

    if "nc" not in _cache:
        _cache["nc"] = _build()
    nc = _cache["nc"]

    in_maps = [
        {"a_shard": np.ascontiguousarray(A_bf[_core_rows(c)])} for c in range(NCORES)
    ]
    res = run_bass_kernel_spmd(
        nc, in_maps, core_ids=list(range(NCORES)), trace=_trace
    )
    _cache["last"] = res
    out = np.empty((N, N), dtype=np.float32)
    for c in range(NCORES):
        shard = res.results[c]["out_shard"]
        for b in range(NST):
            out[b * 2048 + c * BR : b * 2048 + (c + 1) * BR] = shard[
                b * BR : (b + 1) * BR
            ]
    return out
